# revision 1
# baseline (speedup 1.0000x reference)
"""Trainium2 Bass kernel for nn_FComb_79319456023150 (dense_cnn).

Per-pixel MLP over a 96^3 volume: four 1x1x1 convs (38->32->32->32->1 channels
with relu between). z is batch-constant, so w1[:, 32:38] @ z folds into the
layer-1 bias and every layer becomes a K=32 channel GEMM.

Sharding: spatial (outermost X axis) across 8 cores, 110592 pixels each.
Weights/biases replicated.

Device layout per core: the host restripes each shard to [128, 27648] = 4
pixel-blocks x 32 channels on partitions, pixels on the free dim. Each layer
is computed with a BLOCK-DIAGONAL [128, 128] weight (4 copies of W^T on the
diagonal), so one full-array float32r matmul per 512-pixel chunk applies the
32x32 GEMM to all 4 pixel blocks at once (1 col/cycle). The final layer
(wl: 1x32) uses one sparse [128, 128] weight per chunk whose outputs land on
contiguous partitions 4c+m; accumulating the chunk matmuls into one PSUM
bank packs a whole super-chunk's output into rows 0..OROWS-1 for a single
cheap evacuation op and batched, affine output DMAs.

Relu+bias rides the mandatory PSUM->SBUF crossing as ONE whole-crossing op
per layer, alternating between ScalarE (activation Relu w/ bias) and VectorE
(fused tensor_scalar add+max) by (s+layer) parity — these two engines are
the throughput bound (fp32-from-PSUM is 1x on both), and whole ops amortize
their fixed per-op cost best while keeping the two engines' dependency
chains decoupled. FOUR independent super-chunk pipelines (s%4), each owning
one 2-bank PSUM slot (the L4 accumulator reuses the slot after relu3 drains
it), keep both engines ~75% busy. Input DMAs ramp up (4 single-sc loads,
then 3-sc batches) so the pipeline starts early; each HWDGE dma_start costs
~0.65us of issuing-sequencer time, hence the batching.
"""

import sys

import numpy as np

if "/opt/trn_rl_repo" not in sys.path:
    sys.path.insert(0, "/opt/trn_rl_repo")

C = 32          # channels per layer
P = 128         # SBUF/PSUM partitions
RG = 4          # pixel blocks stacked on the partition dim (128/32)
NCHUNK = 2      # 512-wide chunks per super-chunk (PSUM big tile = 2 banks)
CH = 512        # chunk width (one PSUM bank of fp32)
SCW = NCHUNK * CH                    # 1536 free-dim columns per super-chunk
VOL = 96 * 96 * 96                   # full volume
NCORES = 8
NPIX = VOL // NCORES                 # 110592 pixels per core
FREE = NPIX // RG                    # 27648 free-dim columns per core
NSC = FREE // SCW                    # 18 super-chunks per core
OROWS = RG * NCHUNK                  # 12 packed output rows per super-chunk
assert FREE % SCW == 0



def _pick_group(nsc, target):
    for g in range(min(target, nsc), 0, -1):
        if nsc % g == 0:
            return g
    return 1


def _build_nc(npix=NPIX, use_f32r=True, stagger=False, mirror=False):
    import concourse.mybir as mybir
    from concourse import bacc
    from concourse.tile import TileContext
    from concourse.tile_rust import add_dep_helper

    f32 = mybir.dt.float32
    f32r = mybir.dt.float32r if use_f32r else mybir.dt.float32
    Alu = mybir.AluOpType
    Act = mybir.ActivationFunctionType

    free = npix // RG
    nsc = free // SCW
    assert free % SCW == 0 and nsc >= 1
    gin = _pick_group(nsc, 3)        # super-chunks per input DMA
    gout = _pick_group(nsc, 27)       # super-chunks per output tile/DMA group

    nc = bacc.Bacc()
    fm = nc.dram_tensor("fm", [P, free], f32r, kind="ExternalInput")
    wst = nc.dram_tensor("wst", [P, (3 + NCHUNK) * P], f32r, kind="ExternalInput")
    bias = nc.dram_tensor("bias", [P, 4], f32, kind="ExternalInput")
    out = nc.dram_tensor("out", [npix], f32, kind="ExternalOutput")

    # out[m*free + s*SCW + c*CH + n] viewed for batched affine stores
    out_r = out.rearrange(
        "(m go g c n) -> m go g c n", m=RG, go=nsc // gout, g=gout, c=NCHUNK, n=CH
    )

    with TileContext(nc) as tc:
        with (
            tc.tile_pool(name="const", bufs=1) as constp,
            tc.tile_pool(name="data", bufs=4) as datap,
            tc.tile_pool(name="acts", bufs=4) as actp,
            tc.tile_pool(name="outs", bufs=2) as outsp,
            tc.tile_pool(name="psb", bufs=1, space="PSUM") as psb,
        ):
            wtile = constp.tile([P, (3 + NCHUNK) * P], f32r)
            nc.sync.dma_start(wtile, wst[:, :])
            btile = constp.tile([P, 4], f32)
            nc.sync.dma_start(btile, bias[:, :])

            # Input DMA groups: first few single-sc loads so the pipeline
            # starts after ~0.5 MB instead of a full multi-sc transfer, then
            # steady-state groups of `gin` super-chunks.
            groups = [1] * min(4, nsc)
            while sum(groups) < nsc:
                groups.append(min(gin, nsc - sum(groups)))
            group_of = []
            for gidx, g in enumerate(groups):
                group_of += [(gidx, len(group_of), g)] * g
            group_starts = {}
            for s_, (gidx, gbase, g) in enumerate(group_of):
                group_starts.setdefault(gidx, (s_, g))

            xbig = None
            xbase = 0
            ob = None
            sc0_gate = None       # sc0's relu2 op, used to stagger stream B
            for s in range(nsc):
                gidx, gbase, gwidth = group_of[s]
                if s == gbase:
                    xbig = datap.tile([P, gwidth * SCW], f32r, tag="x")
                    xbase = gbase
                    nc.sync.dma_start(
                        xbig, fm[:, gbase * SCW:(gbase + gwidth) * SCW]
                    )
                si = s - xbase
                h = xbig[:, si * SCW:(si + 1) * SCW]

                # Four independent sc streams (s%4), each owning one
                # 2-bank PSUM slot: within a stream, relu(l) must complete
                # before mm(l+1) anyway, so one slot costs nothing, while
                # the streams interleave freely on every engine.
                for layer in range(3):
                    ps = psb.tile([P, SCW], f32, tag=f"ps{s % 4}")
                    wsl = wtile[:, layer * P:(layer + 1) * P]
                    for cc in range(NCHUNK):
                        mm = nc.tensor.matmul(
                            ps[:, cc * CH:(cc + 1) * CH],
                            wsl,
                            h[:, cc * CH:(cc + 1) * CH],
                            start=True,
                            stop=True,
                        )
                        NAME_INFO[mm.ins.name] = (s, f"mm{layer}.{cc}")
                        if stagger and s == 1 and layer == 0 and cc == 0 \
                                and sc0_gate is not None:
                            add_dep_helper(sc0_gate, mm.ins,
                                           reason="stagger stream B")
                    hn = actp.tile([P, SCW], f32r, tag=f"h{layer}")
                    bcol = btile[:, layer:layer + 1]
                    # Whole-crossing relu on one engine, alternating by
                    # (s + layer): each crossing is a single large op (best
                    # per-op amortization) and the two engines' dependency
                    # chains stay decoupled across layers.
                    if (s + layer) % 2 == 0:
                        xop = nc.scalar.activation(
                            hn[:, :], ps[:, :], Act.Relu,
                            bias=bcol, scale=1.0,
                        )
                        NAME_INFO[xop.ins.name] = (s, f"reluA{layer}")
                    else:
                        xop = nc.vector.tensor_scalar(
                            hn[:, :], ps[:, :],
                            bcol, 0.0, Alu.add, Alu.max,
                        )
                        NAME_INFO[xop.ins.name] = (s, f"reluD{layer}")
                    if s == 0 and layer == 1:
                        sc0_gate = xop.ins
                    h = hn

                # Layer 4: chunk c's [128,128] weight has wl only in columns
                # 4c+m (m<4); accumulating the 3 chunk matmuls into one bank
                # leaves out[4c+m, n] = wl @ (block m of chunk c) on the
                # contiguous partitions 0..11.
                go, so = divmod(s, gout)
                if so == 0:
                    ob = outsp.tile([OROWS, gout * CH], f32, tag="ob")
                ps4 = psb.tile([P, CH], f32, tag=f"ps{s % 4}")
                for cc in range(NCHUNK):
                    mm4 = nc.tensor.matmul(
                        ps4[:, :],
                        wtile[:, (3 + cc) * P:(4 + cc) * P],
                        h[:, cc * CH:(cc + 1) * CH],
                        start=(cc == 0),
                        stop=(cc == NCHUNK - 1),
                    )
                    NAME_INFO[mm4.ins.name] = (s, f"mm4.{cc}")
                blcol = btile[:OROWS, 3:4]
                if s % 2 == 0 and s % 8 != 0:
                    fin = nc.vector.tensor_scalar(
                        ob[:, so * CH:(so + 1) * CH], ps4[:OROWS, :],
                        blcol, None, Alu.add,
                    )
                else:
                    fin = nc.scalar.activation(
                        ob[:, so * CH:(so + 1) * CH], ps4[:OROWS, :],
                        Act.Identity, bias=blcol, scale=1.0,
                    )
                NAME_INFO[fin.ins.name] = (s, "final")
                # Store in two waves: the first ~2/3 of the output ships
                # while compute continues, so only the last third's DMA sits
                # in the drain tail.
                if gout == nsc:
                    wsplit = max(1, 8 * nsc // 9)
                    waves = {wsplit - 1: (0, wsplit), nsc - 1: (wsplit, nsc)}
                    if s in waves:
                        a, b = waves[s]
                        for cc in range(NCHUNK):
                            nc.sync.dma_start(
                                out_r[:, 0, a:b, cc, :],
                                ob[RG * cc:RG * cc + RG, a * CH:b * CH].rearrange(
                                    "m (g n) -> m g n", n=CH
                                ),
                            )
                elif so == gout - 1:
                    for cc in range(NCHUNK):
                        nc.sync.dma_start(
                            out_r[:, go, :, cc, :],
                            ob[RG * cc:RG * cc + RG, :].rearrange(
                                "m (g n) -> m g n", n=CH
                            ),
                        )

    # Walrus codegen cannot reliably attach semaphore waits to self-loading
    # matmuls; hoist every matmul's waits onto a PE nop inserted just before
    # it (sequencer-side wait, same semantics).
    for blk in nc.main_func.blocks:
        insts = blk.instructions
        idx = 0
        while idx < len(insts):
            inst = insts[idx]
            if isinstance(inst, mybir.InstMatmult):
                si = inst.sync_info
                if si is not None and len(si.on_wait) > 0:
                    nop = mybir.InstNoOp(
                        name=nc.get_next_instruction_name(), ins=[], outs=[]
                    )
                    nop.engine = inst.engine
                    nop.bass_nofuse = True
                    nop.sync_info = mybir.SyncInfo(on_wait=si.on_wait, on_update=[])
                    si.on_wait = []
                    nc.register_instruction(nop)
                    insts.insert(idx, nop)
                    idx += 1
            idx += 1

    for blk in nc.main_func.blocks:
        for inst in blk.instructions:
            if isinstance(inst, mybir.InstMatmult):
                si = inst.sync_info
                assert si is None or len(si.on_wait) == 0, inst.name

    nc.compile()
    return nc


def _blockdiag4(wT):
    """[32, 32] -> [128, 128] block-diagonal with 4 copies."""
    out = np.zeros((P, P), dtype=np.float32)
    for b in range(RG):
        out[32 * b:32 * b + 32, 32 * b:32 * b + 32] = wT
    return out


def _prep_host_inputs(z, w1, b1, w2, b2, w3, b3, wl, bl):
    """Fold z into the layer-1 bias and build the device weight layouts."""
    f32 = np.float32
    b1e = (b1 + w1[:, C:] @ z[0]).astype(f32)          # [32]

    w4 = np.zeros((P, NCHUNK * P), dtype=f32)
    for cc in range(NCHUNK):
        for m in range(RG):
            w4[32 * m:32 * m + 32, cc * P + RG * cc + m] = wl[0, :]

    wst = np.concatenate(
        [
            _blockdiag4(w1[:, :C].T),
            _blockdiag4(w2.T),
            _blockdiag4(w3.T),
            w4,
        ],
        axis=1,
    ).astype(f32)                                       # [128, 768]

    bias = np.zeros((P, 4), dtype=f32)
    bias[:, 0] = np.tile(b1e, RG)
    bias[:, 1] = np.tile(b2.astype(f32), RG)
    bias[:, 2] = np.tile(b3.astype(f32), RG)
    bias[:, 3] = f32(bl[0])
    return wst, bias


def _restripe(shard):
    """[32, npix] channel-major shard -> [128, npix/4] (block, channel) rows."""
    npix = shard.shape[1]
    return np.ascontiguousarray(
        shard.reshape(C, RG, npix // RG).transpose(1, 0, 2).reshape(P, npix // RG)
    )


_NC_CACHE = {}
NAME_INFO = {}   # instruction name -> (sc, stage) for profiling


def _run(feature_map, z, w1, b1, w2, b2, w3, b3, wl, bl, **spmd_kwargs):
    from concourse.bass_utils import run_bass_kernel_spmd

    feature_map = np.asarray(feature_map, dtype=np.float32)
    z = np.asarray(z, dtype=np.float32)
    w1, b1 = np.asarray(w1, np.float32), np.asarray(b1, np.float32)
    w2, b2 = np.asarray(w2, np.float32), np.asarray(b2, np.float32)
    w3, b3 = np.asarray(w3, np.float32), np.asarray(b3, np.float32)
    wl, bl = np.asarray(wl, np.float32), np.asarray(bl, np.float32)

    wst, bias = _prep_host_inputs(z, w1, b1, w2, b2, w3, b3, wl, bl)

    fm_flat = feature_map.reshape(C, VOL)
    in_maps = []
    for k in range(NCORES):
        shard = _restripe(fm_flat[:, k * NPIX:(k + 1) * NPIX])
        in_maps.append({"fm": shard, "wst": wst, "bias": bias})

    if "nc" not in _NC_CACHE:
        _NC_CACHE["nc"] = _build_nc()
    nc = _NC_CACHE["nc"]

    res = run_bass_kernel_spmd(nc, in_maps, core_ids=list(range(NCORES)), **spmd_kwargs)
    out = np.empty((VOL,), dtype=np.float32)
    for k in range(NCORES):
        out[k * NPIX:(k + 1) * NPIX] = res.results[k]["out"]
    return out.reshape(1, 1, 96, 96, 96), res


def kernel(feature_map, z, w1, b1, w2, b2, w3, b3, wl, bl):
    out, _ = _run(feature_map, z, w1, b1, w2, b2, w3, b3, wl, bl)
    return out



# revision 20
# speedup vs baseline: 1.0696x; 1.0696x over previous
"""Trainium2 Bass kernel for nn_FComb_79319456023150 (dense_cnn).

Per-pixel MLP over a 96^3 volume: four 1x1x1 convs (38->32->32->32->1 channels
with relu between). z is batch-constant, so w1[:, 32:38] @ z folds into the
layer-1 bias and every layer becomes a K=32 channel GEMM.

Sharding: spatial (outermost X axis) across 8 cores, 110592 pixels each.
Weights/biases replicated.

Device layout per core: the host restripes each shard to [128, 27648] = 4
pixel-blocks x 32 channels on partitions, pixels on the free dim, in bf16
(halves input HBM traffic; tolerance is 2e-2 and bf16 moving data costs
~0.3% max error). Each layer is a BLOCK-DIAGONAL [128, 128] bf16 weight
(4 copies of W^T on the diagonal), so one full-array matmul per 512-pixel
chunk applies the 32x32 GEMM to all 4 pixel blocks at once (1 col/cycle,
fp32 PSUM accumulate). The final layer (wl: 1x32) uses one sparse [128, 128]
weight per chunk whose outputs land on partitions 4c+m.

Relu+bias rides the mandatory PSUM->SBUF crossing as ONE whole-crossing op
per layer on ScalarE (activation Relu w/ bias) or VectorE (fused
tensor_scalar add+max) — these two engines are the throughput bound (only
they can read PSUM). Ops are assigned greedily by projected engine busy
(Act 0.833 ns/el vs DVE 1.042 ns/el + per-op overheads) so both engines
stay balanced. FOUR independent super-chunk pipelines (s%4), each owning
one 2-bank PSUM slot (the L4 accumulator reuses the slot after relu3
drains it). Input DMAs ramp up (singles, then multi-sc batches).
"""

import sys

import numpy as np

if "/opt/trn_rl_repo" not in sys.path:
    sys.path.insert(0, "/opt/trn_rl_repo")

C = 32          # channels per layer
P = 128         # SBUF/PSUM partitions
RG = 4          # pixel blocks stacked on the partition dim (128/32)
NCHUNK = 2      # 512-wide chunks per super-chunk (PSUM big tile = 2 banks)
CH = 512        # chunk width (one PSUM bank of fp32)
SCW = NCHUNK * CH                    # 1024 free-dim columns per super-chunk
VOL = 96 * 96 * 96                   # full volume
NCORES = 8
NPIX = VOL // NCORES                 # 110592 pixels per core
FREE = NPIX // RG                    # 27648 free-dim columns per core
NSC = FREE // SCW                    # 27 super-chunks per core
NF = 8                               # L4 fragment matmuls per super-chunk
FW = SCW // NF                       # 128 columns per fragment
OROWS = RG * NF                      # 32 packed output rows (8m+f)
assert FREE % SCW == 0

# Cost-model constants for greedy engine balancing (ns)
_DVE_BIG = 1024 * 1.0417 + 125
_ACT_BIG = 1024 * 0.8333 + 185
_DVE_FIN = FW * 1.0417 + 125
_ACT_FIN = FW * 0.8333 + 185


def _pick_group(nsc, target):
    for g in range(min(target, nsc), 0, -1):
        if nsc % g == 0:
            return g
    return 1


def _build_nc(npix=NPIX, policy="parity"):
    import concourse.mybir as mybir
    from concourse import bacc
    from concourse.tile import TileContext
    from concourse.tile_rust import add_dep_helper

    f32 = mybir.dt.float32
    bf16 = mybir.dt.bfloat16
    Alu = mybir.AluOpType
    Act = mybir.ActivationFunctionType

    free = npix // RG
    nsc = free // SCW
    assert free % SCW == 0 and nsc >= 1
    gin = _pick_group(nsc, 3)        # super-chunks per input DMA
    gout = _pick_group(nsc, 27)      # super-chunks per output tile/DMA group

    nc = bacc.Bacc()
    fm = nc.dram_tensor("fm", [P, free], bf16, kind="ExternalInput")
    wst = nc.dram_tensor("wst", [P, 3 * P + NF * C], bf16, kind="ExternalInput")
    bias = nc.dram_tensor("bias", [P, 4], f32, kind="ExternalInput")
    out = nc.dram_tensor("out", [npix], f32, kind="ExternalOutput")

    # Device output layout: out[p, s, n] with p = 8m+f (block m, fragment f)
    # on partitions, sc index s, fragment column n. The host un-permutes —
    # this makes every output wave a single 3-dim DMA with 512B runs.
    out_r = out.rearrange("(p s n) -> p s n", p=OROWS, s=nsc, n=FW)

    # Eviction-engine assignment: per-sc alternating patterns (A,D,A) or
    # (D,A,D) chosen greedily on cumulative projected busy — keeps the two
    # queues interleaved within every sc while balancing global load.
    # Finals are likewise greedy (DVE 258 vs Act 292 ns).
    busy = {"DVE": 0.0, "ACT": 0.0}
    cur = {"pat": ("ACT", "DVE", "ACT")}

    def pick_engine(dve_cost, act_cost, s=0, layer=0):
        if layer == 3:
            e = "DVE" if busy["DVE"] + dve_cost <= busy["ACT"] + act_cost else "ACT"
            busy[e] += dve_cost if e == "DVE" else act_cost
            return e
        if layer == 0:
            pat_a_act = busy["ACT"] + 2 * act_cost
            pat_a_dve = busy["DVE"] + dve_cost
            pat_d_act = busy["ACT"] + act_cost
            pat_d_dve = busy["DVE"] + 2 * dve_cost
            if max(pat_a_act, pat_a_dve) <= max(pat_d_act, pat_d_dve):
                cur["pat"] = ("ACT", "DVE", "ACT")
            else:
                cur["pat"] = ("DVE", "ACT", "DVE")
        e = cur["pat"][layer]
        busy[e] += dve_cost if e == "DVE" else act_cost
        return e

    with TileContext(nc) as tc:
        with (
            tc.tile_pool(name="const", bufs=1) as constp,
            tc.tile_pool(name="data", bufs=6) as datap,
            tc.tile_pool(name="acts", bufs=4) as actp,
            tc.tile_pool(name="outs", bufs=2) as outsp,
            tc.tile_pool(name="psb", bufs=1, space="PSUM") as psb,
        ):
            # Lead-in: the first half-chunk of sc0 ships before the weights
            # so mm0.c0's moving data and the (smaller) weight DMA pipeline
            # through the serialized DMA engines with minimal latency.
            x0 = datap.tile([P, SCW], bf16, tag="x")
            nc.sync.dma_start(x0[:, :CH], fm[:, :CH])
            wtile = constp.tile([P, (3 + NCHUNK) * P], bf16)
            nc.sync.dma_start(wtile, wst[:, :])
            nc.sync.dma_start(x0[:, CH:], fm[:, CH:SCW])
            btile = constp.tile([P, 4], f32)
            nc.sync.dma_start(btile, bias[:, :])

            # Input DMA groups: a few more single-sc loads so the pipeline
            # starts early, then steady-state groups of `gin` super-chunks.
            groups = [1] * min(4, nsc)
            while sum(groups) < nsc:
                groups.append(min(gin, nsc - sum(groups)))
            group_of = []
            for gidx, g in enumerate(groups):
                group_of += [(gidx, len(group_of), g)] * g

            xbig = None
            xbase = 0
            ob = None
            for s in range(nsc):
                gidx, gbase, gwidth = group_of[s]
                if s == 0:
                    xbig, xbase = x0, 0
                elif s == gbase:
                    xbig = datap.tile([P, gwidth * SCW], bf16, tag="x")
                    xbase = gbase
                    nc.sync.dma_start(
                        xbig, fm[:, gbase * SCW:(gbase + gwidth) * SCW]
                    )
                si = s - xbase
                h = xbig[:, si * SCW:(si + 1) * SCW]

                # Four independent sc streams (s%4), each owning one
                # 2-bank PSUM slot: within a stream, relu(l) must complete
                # before mm(l+1) anyway, so one slot costs nothing, while
                # the streams interleave freely on every engine.
                for layer in range(3):
                    ps = psb.tile([P, SCW], f32, tag=f"ps{s % 4}")
                    wsl = wtile[:, layer * P:(layer + 1) * P]
                    for cc in range(NCHUNK):
                        mm = nc.tensor.matmul(
                            ps[:, cc * CH:(cc + 1) * CH],
                            wsl,
                            h[:, cc * CH:(cc + 1) * CH],
                            start=True,
                            stop=True,
                        )
                        NAME_INFO[mm.ins.name] = (s, f"mm{layer}.{cc}")
                    hn = actp.tile([P, SCW], bf16, tag=f"h{layer}")
                    bcol = btile[:, layer:layer + 1]
                    # Whole-crossing relu on one engine, chosen greedily by
                    # projected busy: each crossing is a single large op
                    # (best per-op amortization).
                    if pick_engine(_DVE_BIG, _ACT_BIG, s, layer) == "ACT":
                        xop = nc.scalar.activation(
                            hn[:, :], ps[:, :], Act.Relu,
                            bias=bcol, scale=1.0,
                        )
                        NAME_INFO[xop.ins.name] = (s, f"reluA{layer}")
                    else:
                        xop = nc.vector.tensor_scalar(
                            hn[:, :], ps[:, :],
                            bcol, 0.0, Alu.add, Alu.max,
                        )
                        NAME_INFO[xop.ins.name] = (s, f"reluD{layer}")
                    h = hn

                # Layer 4 as NF fragment matmuls: fragment f's [128, 32]
                # stationary has wl in column 8m+f (rows 32m..32m+32), so
                # block m of fragment f lands on PSUM row 8m+f. Fragments
                # accumulate into one [32, FW] region (disjoint rows; the
                # zero-contributions add harmlessly), shrinking the final
                # eviction to a 128-element-free op.
                go, so = divmod(s, gout)
                if so == 0:
                    ob = outsp.tile([OROWS, gout * FW], f32, tag="ob")
                ps4 = psb.tile([OROWS, FW], f32, tag=f"ps{s % 4}")
                for f in range(NF):
                    mm4 = nc.tensor.matmul(
                        ps4[:, :],
                        wtile[:, 3 * P + f * C:3 * P + (f + 1) * C],
                        h[:, f * FW:(f + 1) * FW],
                        start=(f == 0),
                        stop=(f == NF - 1),
                    )
                    NAME_INFO[mm4.ins.name] = (s, f"mm4.{f}")
                blcol = btile[:OROWS, 3:4]
                if pick_engine(_DVE_FIN, _ACT_FIN, s, 3) == "DVE":
                    fin = nc.vector.tensor_scalar(
                        ob[:, so * FW:(so + 1) * FW], ps4[:, :],
                        blcol, None, Alu.add,
                    )
                else:
                    fin = nc.scalar.activation(
                        ob[:, so * FW:(so + 1) * FW], ps4[:, :],
                        Act.Identity, bias=blcol, scale=1.0,
                    )
                NAME_INFO[fin.ins.name] = (s, "final")
                # Store in waves: the bulk ships while compute continues and
                # the last wave covers the tail scs; it rides ScalarE's HWDGE
                # queue so it doesn't serialize behind SP-issued input DMAs.
                if gout == nsc:
                    wsplit = max(1, 8 * nsc // 9)
                    waves = {wsplit - 1: (0, wsplit), nsc - 1: (wsplit, nsc)}
                    if s in waves:
                        a, b = waves[s]
                        eng = nc.scalar if s == nsc - 1 else nc.sync
                        eng.dma_start(
                            out_r[:, a:b, :],
                            ob[:, a * FW:b * FW].rearrange(
                                "p (g n) -> p g n", n=FW
                            ),
                        )
                elif so == gout - 1:
                    nc.sync.dma_start(
                        out_r[:, go * gout:(go + 1) * gout, :],
                        ob[:, :].rearrange("p (g n) -> p g n", n=FW),
                    )

    # Walrus codegen cannot reliably attach semaphore waits to self-loading
    # matmuls; hoist every matmul's waits onto a PE nop inserted just before
    # it (sequencer-side wait, same semantics).
    for blk in nc.main_func.blocks:
        insts = blk.instructions
        idx = 0
        while idx < len(insts):
            inst = insts[idx]
            if isinstance(inst, mybir.InstMatmult):
                si = inst.sync_info
                if si is not None and len(si.on_wait) > 0:
                    nop = mybir.InstNoOp(
                        name=nc.get_next_instruction_name(), ins=[], outs=[]
                    )
                    nop.engine = inst.engine
                    nop.bass_nofuse = True
                    nop.sync_info = mybir.SyncInfo(on_wait=si.on_wait, on_update=[])
                    si.on_wait = []
                    nc.register_instruction(nop)
                    insts.insert(idx, nop)
                    idx += 1
            idx += 1

    for blk in nc.main_func.blocks:
        for inst in blk.instructions:
            if isinstance(inst, mybir.InstMatmult):
                si = inst.sync_info
                assert si is None or len(si.on_wait) == 0, inst.name

    nc.compile()
    return nc


def _blockdiag4(wT):
    """[32, 32] -> [128, 128] block-diagonal with 4 copies."""
    out = np.zeros((P, P), dtype=np.float32)
    for b in range(RG):
        out[32 * b:32 * b + 32, 32 * b:32 * b + 32] = wT
    return out


def _prep_host_inputs(z, w1, b1, w2, b2, w3, b3, wl, bl):
    """Fold z into the layer-1 bias and build the device weight layouts."""
    import ml_dtypes

    f32 = np.float32
    b1e = (b1 + w1[:, C:] @ z[0]).astype(f32)          # [32]

    # L4 fragment stationaries: fragment f's [128, 32] block has wl in
    # column 8m+f, rows 32m..32m+32 -> psum row 8m+f gets block m's dot.
    w4 = np.zeros((P, NF * C), dtype=f32)
    for f in range(NF):
        for m in range(RG):
            w4[32 * m:32 * m + 32, f * C + NF * m + f] = wl[0, :]

    wst = np.concatenate(
        [
            _blockdiag4(w1[:, :C].T),
            _blockdiag4(w2.T),
            _blockdiag4(w3.T),
            w4,
        ],
        axis=1,
    ).astype(ml_dtypes.bfloat16)                        # [128, 640]

    bias = np.zeros((P, 4), dtype=f32)
    bias[:, 0] = np.tile(b1e, RG)
    bias[:, 1] = np.tile(b2.astype(f32), RG)
    bias[:, 2] = np.tile(b3.astype(f32), RG)
    bias[:, 3] = f32(bl[0])
    return wst, bias


def _restripe(shard):
    """[32, npix] channel-major bf16 shard -> [128, npix/4] (block, channel)."""
    npix = shard.shape[1]
    return np.ascontiguousarray(
        shard.reshape(C, RG, npix // RG).transpose(1, 0, 2).reshape(P, npix // RG)
    )


_NC_CACHE = {}
NAME_INFO = {}   # instruction name -> (sc, stage) for profiling


def _run(feature_map, z, w1, b1, w2, b2, w3, b3, wl, bl, **spmd_kwargs):
    import ml_dtypes
    from concourse.bass_utils import run_bass_kernel_spmd

    feature_map = np.asarray(feature_map, dtype=np.float32)
    z = np.asarray(z, dtype=np.float32)
    w1, b1 = np.asarray(w1, np.float32), np.asarray(b1, np.float32)
    w2, b2 = np.asarray(w2, np.float32), np.asarray(b2, np.float32)
    w3, b3 = np.asarray(w3, np.float32), np.asarray(b3, np.float32)
    wl, bl = np.asarray(wl, np.float32), np.asarray(bl, np.float32)

    wst, bias = _prep_host_inputs(z, w1, b1, w2, b2, w3, b3, wl, bl)

    fm_flat = feature_map.reshape(C, VOL).astype(ml_dtypes.bfloat16)
    in_maps = []
    for k in range(NCORES):
        shard = _restripe(fm_flat[:, k * NPIX:(k + 1) * NPIX])
        in_maps.append({"fm": shard, "wst": wst, "bias": bias})

    if "nc" not in _NC_CACHE:
        _NC_CACHE["nc"] = _build_nc()
    nc = _NC_CACHE["nc"]

    res = run_bass_kernel_spmd(nc, in_maps, core_ids=list(range(NCORES)), **spmd_kwargs)
    out = np.empty((VOL,), dtype=np.float32)
    for k in range(NCORES):
        # Device layout [p=8m+f, s, n] -> shard-linear m*FREE + s*SCW + f*FW + n
        dev = res.results[k]["out"].reshape(RG, NF, NSC, FW)
        out[k * NPIX:(k + 1) * NPIX] = (
            dev.transpose(0, 2, 1, 3).reshape(NPIX)
        )
    return out.reshape(1, 1, 96, 96, 96), res


def kernel(feature_map, z, w1, b1, w2, b2, w3, b3, wl, bl):
    out, _ = _run(feature_map, z, w1, b1, w2, b2, w3, b3, wl, bl)
    return out


# revision 34
# speedup vs baseline: 1.0794x; 1.0092x over previous
"""Trainium2 Bass kernel for nn_FComb_79319456023150 (dense_cnn).

Per-pixel MLP over a 96^3 volume: four 1x1x1 convs (38->32->32->32->1 channels
with relu between). z is batch-constant, so w1[:, 32:38] @ z folds into the
layer-1 bias and every layer becomes a K=32 channel GEMM.

Sharding: spatial (outermost X axis) across 8 cores, 110592 pixels each.
Weights/biases replicated.

Device layout per core: the host restripes each shard to [128, 27648] = 4
pixel-blocks x 32 channels on partitions, pixels on the free dim, in bf16
(halves input HBM traffic; tolerance is 2e-2 and bf16 moving data costs
~0.3% max error). Each layer is a BLOCK-DIAGONAL [128, 128] bf16 weight
(4 copies of W^T on the diagonal), so one full-array matmul per 512-pixel
chunk applies the 32x32 GEMM to all 4 pixel blocks at once (1 col/cycle,
fp32 PSUM accumulate). The final layer (wl: 1x32) uses one sparse [128, 128]
weight per chunk whose outputs land on partitions 4c+m.

Relu+bias rides the mandatory PSUM->SBUF crossing as ONE whole-crossing op
per layer on ScalarE (activation Relu w/ bias) or VectorE (fused
tensor_scalar add+max) — these two engines are the throughput bound (only
they can read PSUM). Ops are assigned greedily by projected engine busy
(Act 0.833 ns/el vs DVE 1.042 ns/el + per-op overheads) so both engines
stay balanced. FOUR independent super-chunk pipelines (s%4), each owning
one 2-bank PSUM slot (the L4 accumulator reuses the slot after relu3
drains it). Input DMAs ramp up (singles, then multi-sc batches).
"""

import sys

import numpy as np

if "/opt/trn_rl_repo" not in sys.path:
    sys.path.insert(0, "/opt/trn_rl_repo")

C = 32          # channels per layer
P = 128         # SBUF/PSUM partitions
RG = 4          # pixel blocks stacked on the partition dim (128/32)
NCHUNK = 2      # 512-wide chunks per super-chunk (PSUM big tile = 2 banks)
CH = 512        # chunk width (one PSUM bank of fp32)
SCW = NCHUNK * CH                    # 1024 free-dim columns per super-chunk
VOL = 96 * 96 * 96                   # full volume
NCORES = 8
NPIX = VOL // NCORES                 # 110592 pixels per core
FREE = NPIX // RG                    # 27648 free-dim columns per core
NSC = FREE // SCW                    # 27 super-chunks per core
NF = 8                               # L4 fragment matmuls per super-chunk
FW = SCW // NF                       # 128 columns per fragment
OROWS = RG * NF                      # 32 packed output rows (8m+f)
assert FREE % SCW == 0

# Cost-model constants for greedy engine balancing (ns)
_DVE_BIG = 1024 * 1.0417 + 125
_ACT_BIG = 1024 * 0.8333 + 185
_DVE_FIN = FW * 1.0417 + 125
_ACT_FIN = FW * 0.8333 + 185


def _pick_group(nsc, target):
    for g in range(min(target, nsc), 0, -1):
        if nsc % g == 0:
            return g
    return 1


def _build_nc(npix=NPIX, policy="parity", assign=None):
    import concourse.mybir as mybir
    from concourse import bacc
    from concourse.tile import TileContext
    from concourse.tile_rust import add_dep_helper

    f32 = mybir.dt.float32
    bf16 = mybir.dt.bfloat16
    Alu = mybir.AluOpType
    Act = mybir.ActivationFunctionType

    free = npix // RG
    nsc = free // SCW
    assert free % SCW == 0 and nsc >= 1
    gin = _pick_group(nsc, 2)        # super-chunks per input DMA
    gout = _pick_group(nsc, 27)      # super-chunks per output tile/DMA group

    nc = bacc.Bacc()
    fm = nc.dram_tensor("fm", [P, free], bf16, kind="ExternalInput")
    wst = nc.dram_tensor("wst", [P, 3 * P + NF * OROWS], bf16, kind="ExternalInput")
    bias = nc.dram_tensor("bias", [P, 4], f32, kind="ExternalInput")
    out = nc.dram_tensor("out", [npix], f32, kind="ExternalOutput")

    # Device output layout: out[p, s, n] with p = 8m+f (block m, fragment f)
    # on partitions, sc index s, fragment column n. The host un-permutes —
    # this makes every output wave a single 3-dim DMA with 512B runs.
    out_r = out.rearrange("(p s n) -> p s n", p=OROWS, s=nsc, n=FW)

    # Eviction-engine assignment: per-sc alternating patterns (A,D,A) or
    # (D,A,D) chosen greedily on cumulative projected busy — keeps the two
    # queues interleaved within every sc while balancing global load.
    # Finals are likewise greedy (DVE 258 vs Act 292 ns).
    busy = {"DVE": 0.0, "ACT": 0.0}
    cur = {"pat": ("ACT", "DVE", "ACT")}
    ASSIGN_LOG.clear()

    def pick_engine(dve_cost, act_cost, s=0, layer=0):
        if assign is not None:
            e = assign[len(ASSIGN_LOG)]
        elif layer == 3:
            e = "DVE" if busy["DVE"] + dve_cost <= busy["ACT"] + act_cost else "ACT"
        else:
            if layer == 0:
                pat_a = max(busy["ACT"] + 2 * act_cost, busy["DVE"] + dve_cost)
                pat_d = max(busy["ACT"] + act_cost, busy["DVE"] + 2 * dve_cost)
                cur["pat"] = (
                    ("ACT", "DVE", "ACT") if pat_a <= pat_d else ("DVE", "ACT", "DVE")
                )
            e = cur["pat"][layer]
        if e == "DVE":
            busy["DVE"] += dve_cost
        elif e == "ACT":
            busy["ACT"] += act_cost
        else:  # split: roughly half each
            busy["DVE"] += dve_cost / 2
            busy["ACT"] += act_cost / 2
        ASSIGN_LOG.append(((s, layer), e))
        return e

    with TileContext(nc) as tc:
        with (
            tc.tile_pool(name="const", bufs=1) as constp,
            tc.tile_pool(name="data", bufs=12) as datap,
            tc.tile_pool(name="acts", bufs=4) as actp,
            tc.tile_pool(name="outs", bufs=2) as outsp,
            tc.tile_pool(name="psb", bufs=1, space="PSUM") as psb,
        ):
            # Lead-in: the first half-chunk of sc0 ships before the weights
            # so mm0.c0's moving data and the (smaller) weight DMA pipeline
            # through the serialized DMA engines with minimal latency.
            x0 = datap.tile([P, SCW], bf16, tag="x")
            nc.sync.dma_start(x0[:, :CH], fm[:, :CH])
            wtile = constp.tile([P, 3 * P + NF * OROWS], bf16)
            nc.sync.dma_start(wtile, wst[:, :])
            nc.sync.dma_start(x0[:, CH:], fm[:, CH:SCW])
            btile = constp.tile([P, 4], f32)
            nc.sync.dma_start(btile, bias[:, :])

            # Input DMA groups: a few more single-sc loads so the pipeline
            # starts early, then steady-state groups of `gin` super-chunks.
            groups = [1] * min(4, nsc)
            while sum(groups) < nsc:
                groups.append(min(gin, nsc - sum(groups)))
            group_of = []
            for gidx, g in enumerate(groups):
                group_of += [(gidx, len(group_of), g)] * g

            xbig = None
            xbase = 0
            ob = None
            for s in range(nsc):
                gidx, gbase, gwidth = group_of[s]
                if s == 0:
                    xbig, xbase = x0, 0
                elif s == gbase:
                    xbig = datap.tile([P, gwidth * SCW], bf16, tag="x")
                    xbase = gbase
                    nc.sync.dma_start(
                        xbig, fm[:, gbase * SCW:(gbase + gwidth) * SCW]
                    )
                si = s - xbase
                h = xbig[:, si * SCW:(si + 1) * SCW]

                # Four independent sc streams (s%4), each owning one
                # 2-bank PSUM slot: within a stream, relu(l) must complete
                # before mm(l+1) anyway, so one slot costs nothing, while
                # the streams interleave freely on every engine.
                for layer in range(3):
                    ps = psb.tile([P, SCW], f32, tag=f"ps{s % 4}")
                    wsl = wtile[:, layer * P:(layer + 1) * P]
                    for cc in range(NCHUNK):
                        mm = nc.tensor.matmul(
                            ps[:, cc * CH:(cc + 1) * CH],
                            wsl,
                            h[:, cc * CH:(cc + 1) * CH],
                            start=True,
                            stop=True,
                        )
                        NAME_INFO[mm.ins.name] = (s, f"mm{layer}.{cc}")
                    hn = actp.tile([P, SCW], bf16, tag=f"h{layer}")
                    bcol = btile[:, layer:layer + 1]
                    # Whole-crossing relu on one engine (best per-op
                    # amortization), or split across both engines ("S") to
                    # halve the chain latency at slightly higher total cost.
                    eng = pick_engine(_DVE_BIG, _ACT_BIG, s, layer)
                    if eng == "S":
                        xop = nc.scalar.activation(
                            hn[:, :CH], ps[:, :CH], Act.Relu,
                            bias=bcol, scale=1.0,
                        )
                        NAME_INFO[xop.ins.name] = (s, f"reluA{layer}")
                        xop = nc.vector.tensor_scalar(
                            hn[:, CH:], ps[:, CH:],
                            bcol, 0.0, Alu.add, Alu.max,
                        )
                        NAME_INFO[xop.ins.name] = (s, f"reluD{layer}")
                    elif eng == "ACT":
                        xop = nc.scalar.activation(
                            hn[:, :], ps[:, :], Act.Relu,
                            bias=bcol, scale=1.0,
                        )
                        NAME_INFO[xop.ins.name] = (s, f"reluA{layer}")
                    else:
                        xop = nc.vector.tensor_scalar(
                            hn[:, :], ps[:, :],
                            bcol, 0.0, Alu.add, Alu.max,
                        )
                        NAME_INFO[xop.ins.name] = (s, f"reluD{layer}")
                    h = hn

                # Layer 4 as NF fragment matmuls: fragment f's [128, 32]
                # stationary has wl in column 8m+f (rows 32m..32m+32), so
                # block m of fragment f lands on PSUM row 8m+f. Fragments
                # accumulate into one [32, FW] region (disjoint rows; the
                # zero-contributions add harmlessly), shrinking the final
                # eviction to a 128-element-free op.
                go, so = divmod(s, gout)
                if so == 0:
                    ob = outsp.tile([OROWS, gout * FW], f32, tag="ob")
                # ps4 lives in bank1 of the stream slot (cols CH..CH+FW):
                # the next sc's L1 chunk-0 matmul (bank0) then only waits on
                # relu3 of this sc, overlapping the mm4+final tail; only
                # chunk-1 (bank1) waits for the final eviction to drain.
                ps4full = psb.tile([OROWS, CH + FW], f32, tag=f"ps{s % 4}")
                ps4 = ps4full[:, CH:CH + FW]
                for f in range(NF):
                    mm4 = nc.tensor.matmul(
                        ps4[:, :],
                        wtile[:, 3 * P + f * OROWS:3 * P + (f + 1) * OROWS],
                        h[:, f * FW:(f + 1) * FW],
                        start=(f == 0),
                        stop=(f == NF - 1),
                    )
                    NAME_INFO[mm4.ins.name] = (s, f"mm4.{f}")
                blcol = btile[:OROWS, 3:4]
                if pick_engine(_DVE_FIN, _ACT_FIN, s, 3) == "DVE":
                    fin = nc.vector.tensor_scalar(
                        ob[:, so * FW:(so + 1) * FW], ps4[:, :],
                        blcol, None, Alu.add,
                    )
                else:
                    fin = nc.scalar.activation(
                        ob[:, so * FW:(so + 1) * FW], ps4[:, :],
                        Act.Identity, bias=blcol, scale=1.0,
                    )
                NAME_INFO[fin.ins.name] = (s, "final")
                # Store in waves: the bulk ships while compute continues and
                # the last wave covers the tail scs; it rides ScalarE's HWDGE
                # queue so it doesn't serialize behind SP-issued input DMAs.
                if gout == nsc:
                    wsplit = max(1, 8 * nsc // 9)
                    waves = {wsplit - 1: (0, wsplit), nsc - 1: (wsplit, nsc)}
                    if s in waves:
                        a, b = waves[s]
                        eng = nc.scalar if s == nsc - 1 else nc.sync
                        eng.dma_start(
                            out_r[:, a:b, :],
                            ob[:, a * FW:b * FW].rearrange(
                                "p (g n) -> p g n", n=FW
                            ),
                        )
                elif so == gout - 1:
                    nc.sync.dma_start(
                        out_r[:, go * gout:(go + 1) * gout, :],
                        ob[:, :].rearrange("p (g n) -> p g n", n=FW),
                    )

    # Walrus codegen cannot reliably attach semaphore waits to self-loading
    # matmuls; hoist every matmul's waits onto a PE nop inserted just before
    # it (sequencer-side wait, same semantics).
    for blk in nc.main_func.blocks:
        insts = blk.instructions
        idx = 0
        while idx < len(insts):
            inst = insts[idx]
            if isinstance(inst, mybir.InstMatmult):
                si = inst.sync_info
                if si is not None and len(si.on_wait) > 0:
                    nop = mybir.InstNoOp(
                        name=nc.get_next_instruction_name(), ins=[], outs=[]
                    )
                    nop.engine = inst.engine
                    nop.bass_nofuse = True
                    nop.sync_info = mybir.SyncInfo(on_wait=si.on_wait, on_update=[])
                    si.on_wait = []
                    nc.register_instruction(nop)
                    insts.insert(idx, nop)
                    idx += 1
            idx += 1

    for blk in nc.main_func.blocks:
        for inst in blk.instructions:
            if isinstance(inst, mybir.InstMatmult):
                si = inst.sync_info
                assert si is None or len(si.on_wait) == 0, inst.name

    nc.compile()
    return nc


def _blockdiag4(wT):
    """[32, 32] -> [128, 128] block-diagonal with 4 copies."""
    out = np.zeros((P, P), dtype=np.float32)
    for b in range(RG):
        out[32 * b:32 * b + 32, 32 * b:32 * b + 32] = wT
    return out


def _prep_host_inputs(z, w1, b1, w2, b2, w3, b3, wl, bl):
    """Fold z into the layer-1 bias and build the device weight layouts."""
    import ml_dtypes

    f32 = np.float32
    b1e = (b1 + w1[:, C:] @ z[0]).astype(f32)          # [32]

    # L4 fragment stationaries: fragment f's [128, 32] block has wl in
    # column 8m+f, rows 32m..32m+32 -> psum row 8m+f gets block m's dot.
    w4 = np.zeros((P, NF * OROWS), dtype=f32)
    for f in range(NF):
        for m in range(RG):
            w4[32 * m:32 * m + 32, f * OROWS + NF * m + f] = wl[0, :]

    wst = np.concatenate(
        [
            _blockdiag4(w1[:, :C].T),
            _blockdiag4(w2.T),
            _blockdiag4(w3.T),
            w4,
        ],
        axis=1,
    ).astype(ml_dtypes.bfloat16)                        # [128, 640]

    bias = np.zeros((P, 4), dtype=f32)
    bias[:, 0] = np.tile(b1e, RG)
    bias[:, 1] = np.tile(b2.astype(f32), RG)
    bias[:, 2] = np.tile(b3.astype(f32), RG)
    bias[:, 3] = f32(bl[0])
    return wst, bias


def _restripe(shard):
    """[32, npix] channel-major bf16 shard -> [128, npix/4] (block, channel)."""
    npix = shard.shape[1]
    return np.ascontiguousarray(
        shard.reshape(C, RG, npix // RG).transpose(1, 0, 2).reshape(P, npix // RG)
    )


_NC_CACHE = {}
NAME_INFO = {}   # instruction name -> (sc, stage) for profiling
ASSIGN_LOG = []  # [(s, layer), engine] in pick order, for schedule search

# Annealed eviction-engine assignment (one char per pick_engine call, in
# emission order: 4 per sc = 3 big crossings + final). Found by local search
# against the TimelineSim cost model; ~0.6us better than the heuristic.
BEST_ASSIGN = (
    "ADADDADAADADADAADADAADADDADAADADADADDADDADADADADDADAADADDAAAADADDDAD"
    "DADAADAAADADDADDADADDAAAADADADADDADDADAD"
)


def _default_assign():
    if BEST_ASSIGN is None:
        return None
    return ["ACT" if ch == "A" else "DVE" for ch in BEST_ASSIGN]


def _run(feature_map, z, w1, b1, w2, b2, w3, b3, wl, bl, **spmd_kwargs):
    import ml_dtypes
    from concourse.bass_utils import run_bass_kernel_spmd

    feature_map = np.asarray(feature_map, dtype=np.float32)
    z = np.asarray(z, dtype=np.float32)
    w1, b1 = np.asarray(w1, np.float32), np.asarray(b1, np.float32)
    w2, b2 = np.asarray(w2, np.float32), np.asarray(b2, np.float32)
    w3, b3 = np.asarray(w3, np.float32), np.asarray(b3, np.float32)
    wl, bl = np.asarray(wl, np.float32), np.asarray(bl, np.float32)

    wst, bias = _prep_host_inputs(z, w1, b1, w2, b2, w3, b3, wl, bl)

    fm_flat = feature_map.reshape(C, VOL).astype(ml_dtypes.bfloat16)
    in_maps = []
    for k in range(NCORES):
        shard = _restripe(fm_flat[:, k * NPIX:(k + 1) * NPIX])
        in_maps.append({"fm": shard, "wst": wst, "bias": bias})

    if "nc" not in _NC_CACHE:
        _NC_CACHE["nc"] = _build_nc(assign=_default_assign())
    nc = _NC_CACHE["nc"]

    res = run_bass_kernel_spmd(nc, in_maps, core_ids=list(range(NCORES)), **spmd_kwargs)
    out = np.empty((VOL,), dtype=np.float32)
    for k in range(NCORES):
        # Device layout [p=8m+f, s, n] -> shard-linear m*FREE + s*SCW + f*FW + n
        dev = res.results[k]["out"].reshape(RG, NF, NSC, FW)
        out[k * NPIX:(k + 1) * NPIX] = (
            dev.transpose(0, 2, 1, 3).reshape(NPIX)
        )
    return out.reshape(1, 1, 96, 96, 96), res


def kernel(feature_map, z, w1, b1, w2, b2, w3, b3, wl, bl):
    out, _ = _run(feature_map, z, w1, b1, w2, b2, w3, b3, wl, bl)
    return out
